# revision 39
# baseline (speedup 1.0000x reference)
"""DAG-LSTM + MLP Trainium2 kernel: wave-parallel schedule + fp8 DoubleRow.

Data-parallel over batch: 4096 rows -> 512 per NeuronCore x 8 cores, no
collectives.  The DAG (pred_idx/pred_mask) is baked into the instruction
stream at trace time; only steps backward-reachable from the output node
run, and independent steps (same dependency wave) are interleaved in
emission order so every engine's queue stays dense.

The LSTM matmuls run in fp8-e4m3 DoubleRow mode: one PE instruction
computes W0.T@r0 + W1.T@r1 (two K=128 tiles) in ~512 cycles, halving the
gate-matmul stream vs fp16.  Pairing is arranged so every rhs pair is
contiguous in one SBUF tile:
  layer-0: (Wih_m | Bias_m) @ (x | ones-row)   +  (Whh_c0 | Whh_c1) @ h_in
  layer-1: (Wih1_c0|Wih1_c1) @ h0  +  (Whh1_c0|Whh1_c1) @ h1_in
           + (0 | Bias1_m) @ (x | ones-row)
The bias rides a free DoubleRow weight slot (row 0 = b, rhs = ones row),
so gate activations are bias-free and ACT processes same-function chunk
PAIRS at 1024 cols/instruction.  h state, h aggregates and x are fp8
(numpy-simulated end-to-end rel err 5.7e-3 vs the 2e-2 budget); the c
path, gate activations and the MLP stay fp16.  MLP relu(x+b) runs on DVE.

Latency details: predecessor h/c sums are emitted incrementally (early
producers pre-summed off the critical chain); for consumers within 2
cells of their last producer the final h-sum is done ON the PE (one
extra DoubleRow per chunk) so no DVE op gates the hop; tanh(c)->h mul
runs at H-chunk-half granularity so the DVE mul of half 0 overlaps the
ACT tanh of half 1; dummy-matmul heartbeats keep the HAM clock gate at
8/8 through the ACT-bound serial tail.
"""

import os
import sys

import numpy as np

for _p in ("/opt/trn_rl_repo",):
    if _p not in sys.path and os.path.isdir(_p):
        sys.path.insert(0, _p)

B, N, P = 4096, 24, 3
IN, H, L = 128, 256, 2
EXTRA, MH, OUT = 128, 512, 1
NCORES = 8
BC = B // NCORES            # 512 batch rows per core
G4 = 4 * H                  # 1024 gate width
NM = G4 // 128              # 8 gate chunks of 128
HC = H // 128               # 2 hidden chunks

_BUILD_CACHE = {}


def _chunk_k(wt: np.ndarray) -> np.ndarray:
    """[K, M] -> [128, (K//128)*M] with col = kchunk*M + m."""
    k, m = wt.shape
    nk = k // 128
    assert nk * 128 == k
    return np.ascontiguousarray(wt.reshape(nk, 128, m).transpose(1, 0, 2).reshape(128, nk * m))


def _dag_schedule(pred_idx: np.ndarray, pred_mask: np.ndarray):
    """Per step: ({slot: mask-weight for written non-zero slots}, cnt)."""
    sched = []
    for i in range(N):
        tot = 0.0
        w = {}
        for p in range(P):
            m = float(pred_mask[i, p])
            if m == 0.0:
                continue
            tot += m
            s = int(pred_idx[i, p])
            if 1 <= s <= i:
                w[s] = w.get(s, 0.0) + m
        cnt = max(tot, 1.0)
        sched.append((w, cnt))
    return sched


def _needed_variants(sched):
    out = set()
    for w, cnt in sched:
        if w and cnt > 1.0:
            out.add(int(round(cnt)))
    return sorted(out)


def _live_steps(sched):
    live = set()
    stack = [N - 1]
    while stack:
        i = stack.pop()
        if i in live:
            continue
        live.add(i)
        for s in sched[i][0]:
            if s - 1 not in live:
                stack.append(s - 1)
    return live


def _cell_order(sched, live_list):
    """Interleaved emission order of (step, layer) cells (greedy max-slack
    merge of the two per-layer wave-ordered queues)."""
    wave = {}
    for i in live_list:
        deps = [wave[s - 1] for s in sched[i][0]]
        wave[i] = (max(deps) + 1) if deps else 0
    l0 = sorted(live_list, key=lambda i: (wave[i], i))
    l1 = list(l0)
    deps = {}
    for i in live_list:
        deps[(i, 0)] = [(s - 1, 0) for s in sched[i][0]]
        deps[(i, 1)] = [(s - 1, 1) for s in sched[i][0]] + [(i, 0)]
    order = []
    pos = {}
    q = {0: l0, 1: l1}
    hd = {0: 0, 1: 0}
    while hd[0] < len(l0) or hd[1] < len(l1):
        best = None
        for lay in (0, 1):
            if hd[lay] >= len(q[lay]):
                continue
            cell = (q[lay][hd[lay]], lay)
            if any(d not in pos for d in deps[cell]):
                continue
            slack = min((len(order) - pos[d] for d in deps[cell]),
                        default=99)
            if best is None or slack > best[0]:
                best = (slack, lay, cell)
        assert best is not None
        _, lay, cell = best
        pos[cell] = len(order)
        order.append(cell)
        hd[lay] += 1
    return order, pos, wave


def _build(pred_idx: np.ndarray, pred_mask: np.ndarray):
    import concourse.bacc as bacc
    import concourse.tile as tile
    import concourse.mybir as mybir

    F8 = mybir.dt.float8e4
    F16 = mybir.dt.float16
    F32 = mybir.dt.float32
    AF = mybir.ActivationFunctionType
    ALU = mybir.AluOpType
    DR = mybir.MatmulPerfMode.DoubleRow

    sched = _dag_schedule(pred_idx, pred_mask)
    live_list = sorted(_live_steps(sched))
    order, pos, wave = _cell_order(sched, live_list)

    def _terms(j):
        out = []
        for s in sorted(sched[j][0]):
            out += [s] * max(int(round(sched[j][0][s])), 1)
        return out

    # Incremental aggregation hooks: terms whose producers land early are
    # pre-summed as soon as the second-to-last early producer finishes, so
    # only ONE add remains on the critical chain after the last producer.
    agg_pre_after = {}            # emission idx -> [(j, l, early_terms)]
    agg_after = {}                # emission idx -> [(j, l, early, late)]
    for (j, l), k in pos.items():
        w = sched[j][0]
        if not w:
            continue
        terms = _terms(j)
        prods = [(pos[(s - 1, l)], s) for s in terms]
        kmax = max(p for p, _ in prods)
        early = [s for p, s in prods if p < kmax]
        late = [s for p, s in prods if p == kmax]
        if len(early) >= 2:
            k2 = max(p for p, s in prods if p < kmax)
            agg_pre_after.setdefault(k2, []).append((j, l, early))
        agg_after.setdefault(kmax, []).append((j, l, early, late))

    nc = bacc.Bacc("TRN2", target_bir_lowering=False, debug=False,
                   enable_asserts=False, num_devices=NCORES)

    # ---- DRAM parameters (per-core, preprocessed on host) -------------------
    # dagsT[i] = [128, 2, BC] fp8: slot 0 = x_i, slot 1 = ones-row (bias rhs)
    d_dagsT = nc.dram_tensor("dagsT", [N, IN, 2, BC], F8, kind="ExternalInput")
    d_featT = nc.dram_tensor("featT", [EXTRA, BC], F16, kind="ExternalInput")
    # wpk0[m] pair = (Wih0_m | Bias0_m-block); wpk1b[m] pair = (0 | Bias1_m)
    d_wpk0 = nc.dram_tensor("wpk0", [128, 2 * NM, 128], F8, kind="ExternalInput")
    d_wpk1b = nc.dram_tensor("wpk1b", [128, 2 * NM, 128], F8, kind="ExternalInput")
    d_wihT1 = nc.dram_tensor("wihT1", [128, 2, G4], F8, kind="ExternalInput")
    d_whhT0 = nc.dram_tensor("whhT0", [128, 2, G4], F8, kind="ExternalInput")
    d_whhT1 = nc.dram_tensor("whhT1", [128, 2, G4], F8, kind="ExternalInput")
    variants = _needed_variants(sched)
    d_whh_v = {}
    for v in variants:
        for l in range(L):
            d_whh_v[(l, v)] = nc.dram_tensor(
                f"whhT{l}_v{v}", [128, 2, G4], F8, kind="ExternalInput")
    d_mw0T = nc.dram_tensor("mw0T", [128, 3 * MH], F16, kind="ExternalInput")
    d_mw1T = nc.dram_tensor("mw1T", [128, 4 * MH], F16, kind="ExternalInput")
    d_mw2T = nc.dram_tensor("mw2T", [128, 4], F16, kind="ExternalInput")
    d_mb0 = nc.dram_tensor("mb0", [128, 4], F32, kind="ExternalInput")
    d_mb1 = nc.dram_tensor("mb1", [128, 4], F32, kind="ExternalInput")
    d_mb2 = nc.dram_tensor("mb2", [128, 1], F32, kind="ExternalInput")
    d_out = nc.dram_tensor("out", [1, BC], F32, kind="ExternalOutput")

    def cell_vkey(i):
        w, cnt = sched[i]
        return int(round(cnt)) if (w and cnt > 1.0) else 1

    with tile.TileContext(nc) as tc:
        from contextlib import ExitStack
        with ExitStack() as ctx:
            wpool = ctx.enter_context(tc.tile_pool(name="weights", bufs=1))
            spool = ctx.enter_context(tc.tile_pool(name="states", bufs=1))
            xpool = ctx.enter_context(tc.tile_pool(name="xin", bufs=1))
            gpool = ctx.enter_context(tc.tile_pool(name="gact", bufs=5))
            kpool = ctx.enter_context(tc.tile_pool(name="work", bufs=3))
            apool = ctx.enter_context(tc.tile_pool(name="agg", bufs=1))
            ppool = ctx.enter_context(tc.tile_pool(name="psum", bufs=3, space="PSUM"))
            hpool = ctx.enter_context(tc.tile_pool(name="hbeat", bufs=1, space="PSUM"))

            def wload(tag, dram, shape, dt, split=1):
                t = wpool.tile(shape, dt, tag=tag)
                if split == 1 or len(shape) != 2:
                    nc.sync.dma_start(out=t[...], in_=dram[...])
                else:
                    step = shape[1] // split
                    for j in range(split):
                        nc.sync.dma_start(out=t[:, j * step:(j + 1) * step],
                                          in_=dram[:, j * step:(j + 1) * step])
                return t

            x_tiles = {}

            def fetch_x(i):
                if i not in x_tiles:
                    t = xpool.tile([128, 2, BC], F8, tag=f"x{i}")
                    nc.sync.dma_start(out=t[...], in_=d_dagsT[i])
                    x_tiles[i] = t

            wpk0 = wload("wpk0", d_wpk0, [128, 2 * NM, 128], F8)
            for i, _l in order[:3]:
                fetch_x(i)
            wihT1 = wload("wihT1", d_wihT1, [128, 2, G4], F8)
            wpk1b = wload("wpk1b", d_wpk1b, [128, 2 * NM, 128], F8)
            whh_load_order = []
            for i, l in order:
                if not sched[i][0]:
                    continue
                key = (l, cell_vkey(i))
                if key not in whh_load_order:
                    whh_load_order.append(key)
            whh_v = {}
            for (l, v) in whh_load_order:
                if v == 1:
                    dram = d_whhT0 if l == 0 else d_whhT1
                    tag = f"whhT{l}"
                else:
                    dram = d_whh_v[(l, v)]
                    tag = f"whhT{l}_v{v}"
                whh_v[(l, v)] = wload(tag, dram, [128, 2, G4], F8)
            for i in live_list:
                fetch_x(i)
            featT = wload("featT", d_featT, [EXTRA, BC], F16)
            mw0T = wload("mw0T", d_mw0T, [128, 3 * MH], F16)
            mw1T = wload("mw1T", d_mw1T, [128, 4 * MH], F16)
            mw2T = wload("mw2T", d_mw2T, [128, 4], F16)
            mb0 = wload("mb0", d_mb0, [128, 4], F32)
            mb1 = wload("mb1", d_mb1, [128, 4], F32)
            mb2 = wload("mb2", d_mb2, [128, 1], F32)

            h_tiles = {}                           # (slot, layer) -> fp8 tile
            c_tiles = {}                           # (slot, layer) -> f16 tile
            agg_tiles = {}

            SIG = AF.Sigmoid
            TANH = AF.Tanh

            agg_pre = {}                       # (j, l) -> (acc_h, acc_c)

            def emit_agg_pre(j, l, early):
                """Sum the early-producer terms off the critical chain."""
                acc_h = apool.tile([128, HC * BC], F8, tag=f"ph{j}_{l}")
                acc_c = apool.tile([128, HC * BC], F16, tag=f"pc{j}_{l}")
                nc.vector.tensor_add(acc_h[:, :],
                                     h_tiles[(early[0], l)][:, :],
                                     h_tiles[(early[1], l)][:, :])
                for s in early[2:]:
                    nc.vector.tensor_add(acc_h[:, :], acc_h[:, :],
                                         h_tiles[(s, l)][:, :])
                nc.gpsimd.tensor_add(acc_c[:, :],
                                     c_tiles[(early[0], l)][:, :],
                                     c_tiles[(early[1], l)][:, :])
                for s in early[2:]:
                    nc.gpsimd.tensor_add(acc_c[:, :], acc_c[:, :],
                                         c_tiles[(s, l)][:, :])
                agg_pre[(j, l)] = (acc_h, acc_c)

            def emit_agg(j, l, early, late, kprod):
                """Finish predecessor sums: at most one h add (on DVE --
                it gates the consumer's matmuls) after the last producer."""
                terms = early + late
                gap = pos[(j, l)] - kprod
                if len(terms) == 1:
                    agg_tiles[(j, l)] = (h_tiles[(terms[0], l)],
                                         c_tiles[(terms[0], l)])
                    return
                ceng = nc.vector if gap <= 2 else nc.gpsimd
                if (j, l) in agg_pre:
                    acc_h, acc_c = agg_pre.pop((j, l))
                    rest = late
                else:
                    acc_h = apool.tile([128, HC * BC], F8, tag=f"ph{j}_{l}")
                    acc_c = apool.tile([128, HC * BC], F16, tag=f"pc{j}_{l}")
                    nc.vector.tensor_add(acc_h[:, :],
                                         h_tiles[(terms[0], l)][:, :],
                                         h_tiles[(terms[1], l)][:, :])
                    ceng.tensor_add(acc_c[:, :],
                                    c_tiles[(terms[0], l)][:, :],
                                    c_tiles[(terms[1], l)][:, :])
                    rest = terms[2:]
                for s in rest:
                    nc.vector.tensor_add(acc_h[:, :], acc_h[:, :],
                                         h_tiles[(s, l)][:, :])
                    ceng.tensor_add(acc_c[:, :], acc_c[:, :],
                                    c_tiles[(s, l)][:, :])
                agg_tiles[(j, l)] = (acc_h, acc_c)

            # PE warmup for the HAM clock gate
            wu_src = kpool.tile([128, BC], F16, tag="wu")
            nc.vector.memset(wu_src[:, :], 0.0)


            def heartbeat(dep_ap, n=2):
                ps = hpool.tile([128, BC], F32, tag="hb")
                for j in range(n):
                    nc.tensor.matmul(ps[:, :], wu_src[:, 0:128], dep_ap,
                                     start=(j == 0), stop=(j == n - 1))

            def emit_cell(i, l):
                w, cnt = sched[i]
                has_pred = bool(w)
                inv = 1.0 / cnt
                vkey = cell_vkey(i)
                whh = whh_v.get((l, vkey))
                xt = x_tiles[i]
                if l == 1:
                    h0 = h_tiles[(i + 1, 0)]
                h_in, c_sum = agg_tiles.pop((i, l), (None, None))

                def pair(t):
                    return t[:, :].rearrange("p (a b) -> p a b", a=2)

                gact = gpool.tile([128, NM * BC], F16, tag="gact")

                def emit_pair(m0):
                    """Gate chunks m0, m0+1 -> one [128,2,BC] psum pair tile,
                    one bias-free 1024-wide ACT."""
                    pt = ppool.tile([128, 2, BC], F32, tag="gp")
                    for mi, m in enumerate((m0, m0 + 1)):
                        out = pt[:, mi, :]
                        group = []
                        if l == 0:
                            group.append((wpk0[:, 2 * m:2 * m + 2, :], xt[...]))
                            if has_pred:
                                group.append(
                                    (whh[:, :, m * 128:(m + 1) * 128],
                                     pair(h_in)))
                        else:
                            group.append(
                                (wihT1[:, :, m * 128:(m + 1) * 128],
                                 pair(h0)))
                            if has_pred:
                                group.append(
                                    (whh[:, :, m * 128:(m + 1) * 128],
                                     pair(h_in)))
                            group.append((wpk1b[:, 2 * m:2 * m + 2, :],
                                          xt[...]))
                        for j, (lhsT, rhs) in enumerate(group):
                            nc.tensor.matmul(out, lhsT, rhs,
                                             start=(j == 0),
                                             stop=(j == len(group) - 1),
                                             perf_mode=DR,
                                             skip_group_check=True)
                    func = TANH if m0 == 4 else SIG
                    nc.scalar.activation(gact[:, m0 * BC:(m0 + 2) * BC],
                                         pt[...], func)

                sigi = gact[:, 0 * BC:2 * BC]
                sigf = gact[:, 2 * BC:4 * BC]
                tg = gact[:, 4 * BC:6 * BC]
                sigo = gact[:, 6 * BC:8 * BC]
                c_new = spool.tile([128, HC * BC], F16, tag=f"c{i + 1}_{l}")
                h_new = spool.tile([128, HC * BC], F8, tag=f"h{i + 1}_{l}")
                th = kpool.tile([128, HC * BC], F16, tag="th")

                if not has_pred:
                    for m0 in (0, 4):
                        emit_pair(m0)
                    nc.vector.tensor_mul(c_new[:, :], sigi, tg)
                    emit_pair(6)
                    nc.scalar.activation(th[:, :], c_new[:, :], TANH)
                    nc.vector.tensor_mul(h_new[:, :], sigo, th[:, :])
                else:
                    emit_pair(2)                   # f first: c path starts
                    if cnt == 1.0:
                        nc.vector.tensor_mul(c_new[:, :], sigf, c_sum[:, :])
                    else:
                        nc.vector.scalar_tensor_tensor(
                            c_new[:, :], c_sum[:, :], inv, sigf,
                            ALU.mult, ALU.mult)
                    for m0 in (0, 4):
                        emit_pair(m0)
                    t2 = kpool.tile([128, HC * BC], F16, tag="t2")
                    nc.vector.tensor_mul(t2[:, :], sigi, tg)
                    nc.vector.tensor_add(c_new[:, :], c_new[:, :], t2[:, :])
                    emit_pair(6)
                    nc.scalar.activation(th[:, :], c_new[:, :], TANH)
                    nc.vector.tensor_mul(h_new[:, :], sigo, th[:, :])

                h_tiles[(i + 1, l)] = h_new
                c_tiles[(i + 1, l)] = c_new
                return gact

            ncells = len(order)
            for k, (i, l) in enumerate(order):
                gact = emit_cell(i, l)

                for (j, l2, early) in agg_pre_after.get(k, []):
                    emit_agg_pre(j, l2, early)
                for (j, l2, early, late) in agg_after.get(k, []):
                    emit_agg(j, l2, early, late, k)

            # ---- MLP (fp16) -----------------------------------------------
            hlast = h_tiles[(N, L - 1)]
            fc_chunks = [hlast[:, 0:BC], hlast[:, BC:2 * BC], featT[:, :]]

            a0 = gpool.tile([128, 4 * BC], F16, tag="gact")
            for mo in range(4):
                pt = ppool.tile([128, 2, BC], F32, tag="gp")
                ps = pt[:, 0, :]
                for j, fch in enumerate(fc_chunks):
                    nc.tensor.matmul(
                        ps,
                        mw0T[:, j * MH + mo * 128: j * MH + (mo + 1) * 128],
                        fch, start=(j == 0), stop=(j == len(fc_chunks) - 1),
                        skip_group_check=True)
                nc.vector.tensor_scalar(a0[:, mo * BC:(mo + 1) * BC],
                                        ps, mb0[:, mo:mo + 1], 0.0,
                                        ALU.add, ALU.max)
                if mo == 0:
                    heartbeat(a0[:, 0:BC])

            a1 = gpool.tile([128, 4 * BC], F16, tag="gact")
            for mo in range(4):
                pt = ppool.tile([128, 2, BC], F32, tag="gp")
                ps = pt[:, 0, :]
                for kc in range(4):
                    nc.tensor.matmul(
                        ps,
                        mw1T[:, kc * MH + mo * 128: kc * MH + (mo + 1) * 128],
                        a0[:, kc * BC:(kc + 1) * BC],
                        start=(kc == 0), stop=(kc == 3),
                        skip_group_check=True)
                nc.vector.tensor_scalar(a1[:, mo * BC:(mo + 1) * BC],
                                        ps, mb1[:, mo:mo + 1], 0.0,
                                        ALU.add, ALU.max)
                if mo == 0:
                    heartbeat(a1[:, 0:BC])

            pt = ppool.tile([128, 2, BC], F32, tag="gp")
            psf = pt[:, 0, :]
            for kc in range(4):
                nc.tensor.matmul(psf[:1, :], mw2T[:, kc:kc + 1],
                                 a1[:, kc * BC:(kc + 1) * BC],
                                 start=(kc == 0), stop=(kc == 3),
                                 skip_group_check=True)
            out_sb = kpool.tile([128, BC], F32, tag="th")
            nc.scalar.activation(out_sb[:1, :], psf[:1, :], AF.Identity,
                                 bias=mb2[:1, 0:1])
            nc.sync.dma_start(out=d_out[:, :], in_=out_sb[:1, :])

    nc.compile()
    return nc


def _prep_core_inputs(inputs):
    """Host-side layout prep shared by all cores + per-core slices."""
    import ml_dtypes
    f8 = ml_dtypes.float8_e4m3
    f16 = np.float16
    f32 = np.float32

    sched = _dag_schedule(np.asarray(inputs["pred_idx"], np.int32),
                          np.asarray(inputs["pred_mask"], np.int32))

    b0 = (inputs["bih0"] + inputs["bhh0"]).astype(f32)     # [G4]
    b1 = (inputs["bih1"] + inputs["bhh1"]).astype(f32)

    wihT0 = _chunk_k(np.ascontiguousarray(inputs["Wih0"].T))   # [128, G4]
    wihT1 = _chunk_k(np.ascontiguousarray(inputs["Wih1"].T))   # [128, 2*G4]
    whhT0f = _chunk_k(np.ascontiguousarray(inputs["Whh0"].T))
    whhT1f = _chunk_k(np.ascontiguousarray(inputs["Whh1"].T))

    def bias_block(b, m):
        blk = np.zeros((128, 128), f32)
        blk[0, :] = b[m * 128:(m + 1) * 128]
        return blk

    # wpk0: [128, 2*NM, 128]: pair m = (Wih0_m, Bias0_m)
    wpk0 = np.zeros((128, 2 * NM, 128), f32)
    wpk1b = np.zeros((128, 2 * NM, 128), f32)
    for m in range(NM):
        wpk0[:, 2 * m, :] = wihT0[:, m * 128:(m + 1) * 128]
        wpk0[:, 2 * m + 1, :] = bias_block(b0, m)
        wpk1b[:, 2 * m + 1, :] = bias_block(b1, m)

    var_arrays = {}
    for v in _needed_variants(sched):
        var_arrays[f"whhT0_v{v}"] = (whhT0f / v).astype(f8).reshape(128, 2, G4)
        var_arrays[f"whhT1_v{v}"] = (whhT1f / v).astype(f8).reshape(128, 2, G4)

    mw0T = _chunk_k(np.ascontiguousarray(inputs["mW0"].T)).astype(f16)
    mw1T = _chunk_k(np.ascontiguousarray(inputs["mW1"].T)).astype(f16)
    mw2T = _chunk_k(np.ascontiguousarray(inputs["mW2"].T)).astype(f16)
    mb0 = np.ascontiguousarray(inputs["mb0"].astype(f32).reshape(4, 128).T)
    mb1 = np.ascontiguousarray(inputs["mb1"].astype(f32).reshape(4, 128).T)
    mb2 = np.zeros((128, 1), f32)
    mb2[0, 0] = np.float32(inputs["mb2"][0])

    shared = dict(wpk0=wpk0.astype(f8), wpk1b=wpk1b.astype(f8),
                  wihT1=wihT1.astype(f8).reshape(128, 2, G4),
                  whhT0=whhT0f.astype(f8).reshape(128, 2, G4),
                  whhT1=whhT1f.astype(f8).reshape(128, 2, G4),
                  mw0T=mw0T, mw1T=mw1T, mw2T=mw2T,
                  mb0=mb0, mb1=mb1, mb2=mb2, **var_arrays)

    dags = np.asarray(inputs["dags"], np.float32)
    feats = np.asarray(inputs["features"], np.float32)
    in_maps = []
    for c in range(NCORES):
        lo, hi = c * BC, (c + 1) * BC
        dagsT = np.zeros((N, IN, 2, BC), f32)
        dagsT[:, :, 0, :] = dags[lo:hi].transpose(1, 2, 0)
        dagsT[:, 0, 1, :] = 1.0                       # ones row (bias rhs)
        featT = np.ascontiguousarray(feats[lo:hi].T).astype(f16)
        m = dict(shared)
        m["dagsT"] = dagsT.astype(f8)
        m["featT"] = featT
        in_maps.append(m)
    return in_maps


def _get_nc(pred_idx, pred_mask):
    key = (pred_idx.tobytes(), pred_mask.tobytes())
    if key not in _BUILD_CACHE:
        _BUILD_CACHE[key] = _build(pred_idx, pred_mask)
    return _BUILD_CACHE[key]


def run(inputs, trace=False):
    from concourse.bass_utils import run_bass_kernel_spmd

    pred_idx = np.asarray(inputs["pred_idx"], np.int32)
    pred_mask = np.asarray(inputs["pred_mask"], np.int32)
    nc = _get_nc(pred_idx, pred_mask)
    in_maps = _prep_core_inputs(inputs)
    res = run_bass_kernel_spmd(nc, in_maps, core_ids=list(range(NCORES)),
                               trace=trace)
    out = np.concatenate([np.asarray(r["out"], np.float32).reshape(BC)
                          for r in res.results])
    return out, res


def kernel(**inputs) -> np.ndarray:
    out, _ = run(inputs, trace=False)
    return out
